# revision 33
# baseline (speedup 1.0000x reference)
"""Trainium2 kernel for nn_MetaLearner: out[n] = F(x_t[n]) pointwise.

The network (1->H linear, 2 stacked LayerNorm-LSTM cells applied once from
zero state, H->1 readout) collapses to a scalar function F: R -> R because
x_t has a single feature. F is analytic and saturates at both tails.

Strategy: build (on host, from the actual weights passed in) a factored
rational approximation

    F(x) ~ C * prod_k fP_k(s) / prod_k fQ_k(s),   s = clip(x/R, -1, 1)

with quadratic factors f(s) = (s+h)^2 + e taken from the poles/zeros of an
AAA rational fit, polished by Gauss-Newton, and validated in exact-fp32
simulation of the device op sequence (target <= ~1.5e-6 max abs error,
which is far inside the fp32 reference envelope of this problem).

On device each quadratic factor costs 1 ScalarE op (Square activation with
free bias: (s+h)^2) + 1 VectorE op (fused (sq+e)*acc scalar_tensor_tensor),
so the two engines run in parallel. The final division uses the ~2-ULP
custom-DVE reciprocal. Everything is done in a single [128, 977] fp32 SBUF
tile per core (125,056 coords); 8 cores cover N=1e6 data-parallel with a
small overlap on the last core (identical values written twice).
"""

import numpy as np

# ---------------------------------------------------------------- reference
# numpy (float64) replica of the reference network, used only on host to
# sample F at fitting/validation nodes. All parameters come from kernel()'s
# inputs at call time - nothing is baked in.

_H = 20
_L = 2
_FG_BIAS = 1.0
_EPS = 1e-5

N_TOTAL = 1_000_000
N_CORES = 8
PART = 128
FREE = 977
PER_CORE = PART * FREE  # 125056


def _ln(x, g, b):
    mu = np.mean(x, axis=-1, keepdims=True)
    var = np.mean((x - mu) ** 2, axis=-1, keepdims=True)
    return (x - mu) / np.sqrt(var + _EPS) * g + b


def _sigmoid(x):
    return 1.0 / (1.0 + np.exp(-x))


def _ref_np(x_t, W1, b1, Wih, Whh, b_ih, b_hh, g_x, be_x, g_h, be_h, g_c, be_c,
            Wo, bo):
    h = x_t @ W1.T + b1
    hx = np.zeros((x_t.shape[0], _H))
    cx = np.zeros((x_t.shape[0], _H))
    for l in range(_L):
        ig = _ln(h @ Wih[l].T, g_x[l], be_x[l])
        hg = _ln(hx @ Whh[l].T, g_h[l], be_h[l])
        gates = ig + hg + b_ih[l] + b_hh[l]
        i, f, g, o = np.split(gates, 4, axis=-1)
        c = _sigmoid(f + _FG_BIAS) * cx + _sigmoid(i) * np.tanh(g)
        h_new = _sigmoid(o) * np.tanh(_ln(c, g_c[l], be_c[l]))
        hx, cx = h_new, c
        h = h_new
    out = h @ Wo.T + bo
    return np.squeeze(out, axis=-1)


# ------------------------------------------------------------ model fitting

def _pair_factors(roots, R):
    """Roots (x-domain) -> quadratic/linear factor params in s = x/R."""
    roots = np.asarray(roots) / R
    quads, reals = [], []
    used = np.zeros(len(roots), bool)
    for i, r in enumerate(roots):
        if used[i]:
            continue
        if abs(r.imag) > 1e-12:
            j = int(np.argmin(np.abs(roots - r.conjugate()) + used * 1e9))
            used[i] = used[j] = True
            h = -r.real
            quads.append((h, abs(r) ** 2 - h ** 2))
        else:
            used[i] = True
            reals.append(r.real)
    factors = [("q", h, e) for h, e in quads]
    reals.sort()
    while len(reals) >= 2:
        r1, r2 = reals.pop(), reals.pop()
        h = -(r1 + r2) / 2
        factors.append(("q", h, r1 * r2 - h * h))
    if reals:
        factors.append(("l", -reals[0]))
    return factors


def _eval_factors(factors, s):
    acc = np.ones_like(s)
    for f in factors:
        acc = acc * (((s + f[1]) ** 2 + f[2]) if f[0] == "q" else (s + f[1]))
    return acc


def _eval_factors_scaled(factors, gains, s):
    acc = np.ones_like(s)
    for f, g in zip(factors, gains):
        acc = acc * (g * (((s + f[1]) ** 2 + f[2]) if f[0] == "q"
                          else (s + f[1])))
    return acc


def _attach_gains(model):
    """Per-factor gains normalizing each quadratic to geometric mean ~1 on
    the interval. On device the gain is free: the ACT Square computes
    (sqrt(g)*s + sqrt(g)*h)^2 via its scale/bias immediates and the DVE adds
    g*e. Without this, products of ~17 normalized quadratics reach 1e-23,
    uncomfortably close to fp32 underflow."""
    sd = np.linspace(-1, 1, 20001)
    logs = {}
    for side in ("fP", "fQ"):
        gains = []
        for f in model[side]:
            if f[0] == "q":
                v = (sd + f[1]) ** 2 + f[2]
                g = float(np.exp(-np.mean(np.log(np.abs(v) + 1e-300))))
            else:
                g = 1.0
            gains.append(g)
        model["g" + side[1]] = gains
        logs[side] = float(np.sum(np.log(gains)))
    # C_dev * (P * prod gP) / (Q * prod gQ) == C * P / Q
    model["C_dev"] = float(model["C"] * np.exp(logs["fQ"] - logs["fP"]))
    return model


def _f32(a):
    return np.asarray(a, np.float32)


def _eval_f32_device(model, x):
    """Exact fp32 simulation of the device op sequence."""
    R, fP, fQ = model["R"], model["fP"], model["fQ"]
    C = model.get("C_dev", model["C"])
    gP = model.get("gP", [1.0] * len(fP))
    gQ = model.get("gQ", [1.0] * len(fQ))
    # clamp in x units; the 1/R normalization rides the ACT Square's scale
    sx = _f32(np.maximum(_f32(np.minimum(_f32(x), _f32(R))), _f32(-R)))
    s = _f32(sx * _f32(1.0 / R))  # for linear factors only

    def chain(factors, gains):
        acc = None
        for f, g in zip(factors, gains):
            if f[0] == "q":
                sg = _f32(np.sqrt(g) / R)
                hb = _f32(np.sqrt(g) * f[1])
                eg = _f32(g * f[2])
                z = _f32(_f32(sg * sx) + hb)
                sq = _f32(z * z)
                t = _f32(sq + eg) if acc is None else _f32(_f32(sq + eg) * acc)
            else:
                h = _f32(f[1])
                t = _f32(s + h) if acc is None else _f32(_f32(s + h) * acc)
            acc = t
        return acc

    P, Q = chain(fP, gP), chain(fQ, gQ)
    if model.get("recip_mode", "native") == "actseed":
        # ACT table seed exp(-ln Q) (modeled at fp32 precision; the real
        # tables are a bit coarser, which the Newton step absorbs) then one
        # Newton iteration in the sign-flipped form the device uses.
        r0 = _f32(np.exp(_f32(-_f32(np.log(Q.astype(np.float64)).astype(
            np.float32))).astype(np.float64)).astype(np.float32))
        t = _f32(Q * r0)
        rm = _f32(_f32(t - _f32(2.0)) * r0)          # == -1/Q
        return _f32(_f32(P * _f32(-C)) * rm)
    r = _f32(_f32(1.0) / Q)
    return _f32(_f32(P * _f32(C)) * r)


def _build_model_once(F, R, rtol, n_samp=6000, polish_iters=8,
                      max_terms=48):
    from scipy.interpolate import AAA

    xs = np.cos(np.pi * (np.arange(n_samp) + 0.5) / n_samp) * R
    ys = F(xs)
    r = AAA(xs, ys, rtol=rtol, max_terms=max_terms)
    poles = list(r.poles())
    zeros = list(r.roots())
    # Annihilate true Froissart doublets (pole ~ coincident zero; genuine
    # branch-cut poles keep a finite separation from the interlaced zeros)
    # and drop far-away singularities. Poles extremely close to the real
    # interval are also rejected as artifacts - F's nearest true
    # singularities sit ~0.17 off the axis for this net.
    keep_p = []
    for p in poles:
        d = [abs(p - z) for z in zeros] or [np.inf]
        j = int(np.argmin(d))
        spurious = (d[j] < 1e-5 * R) or (
            abs(p.imag) < 2e-3 * R and abs(p.real) < 1.2 * R
            and d[j] < 0.1 * R)
        if spurious and np.isfinite(d[j]):
            zeros.pop(j)
        else:
            keep_p.append(p)
    keep_p = [p for p in keep_p if abs(p) < 50 * R]
    zeros = [z for z in zeros if abs(z) < 50 * R]
    fQ = _pair_factors(keep_p, R)
    fP = _pair_factors(zeros, R)

    dense = np.linspace(-R, R, 200001)
    Fd = F(dense)
    sd = dense / R

    Pv = _eval_factors(fP, sd)
    Qv = _eval_factors(fQ, sd)
    ratio = Pv / Qv
    C = float(np.dot(ratio, Fd) / np.dot(ratio, ratio))

    # Gauss-Newton polish of (log|C|, all h/e params) on a subsampled grid
    sgnC = np.sign(C) or 1.0

    def pack(C_, fP_, fQ_):
        th = [np.log(abs(C_))]
        for f in fP_ + fQ_:
            th += [f[1], f[2]] if f[0] == "q" else [f[1]]
        return np.array(th)

    nP = len(fP)

    def unpack(th):
        C_ = sgnC * np.exp(th[0])
        i = 1
        out = []
        for f in fP + fQ:
            if f[0] == "q":
                out.append(("q", th[i], th[i + 1])); i += 2
            else:
                out.append(("l", th[i])); i += 1
        return C_, out[:nP], out[nP:]

    th = pack(C, fP, fQ)

    def resid(th_):
        C_, fP_, fQ_ = unpack(th_)
        return C_ * _eval_factors(fP_, sd) / _eval_factors(fQ_, sd) - Fd

    lam = 1e-6
    rb = resid(th)
    sub = slice(None, None, 20)
    for _ in range(polish_iters):
        r0 = resid(th)[sub]
        J = np.empty((r0.size, th.size))
        for k in range(th.size):
            dt = 1e-6 * max(1.0, abs(th[k]))
            t2 = th.copy(); t2[k] += dt
            J[:, k] = (resid(t2)[sub] - r0) / dt
        g = J.T @ r0
        try:
            step = np.linalg.solve(J.T @ J + lam * np.eye(th.size), g)
        except np.linalg.LinAlgError:
            break
        t2 = th - step
        r2 = resid(t2)
        if np.sqrt((r2 ** 2).mean()) < np.sqrt((rb ** 2).mean()):
            th, rb = t2, r2
            lam = max(lam * 0.5, 1e-9)
        else:
            lam *= 4
            if lam > 1e3:
                break
    C, fP, fQ = unpack(th)
    return {"R": R, "C": float(C), "fP": fP, "fQ": fQ}


def build_model(weights):
    """weights: dict of float64 numpy arrays (all inputs except x_t)."""
    def F(xs):
        return _ref_np(np.asarray(xs, np.float64).reshape(-1, 1), **weights)

    R = 7.0
    dense = np.linspace(-R, R, 200001)
    Fd = F(dense)
    rng = np.random.default_rng(12345)
    xn = np.clip(rng.normal(size=300000), -R, R)
    Fn = F(xn)

    best = None          # (err, nops, model) minimizing err
    cheapest = None      # same, but min nops among err <= ACCEPT
    ACCEPT = 1.6e-6
    # ordered cheapest-first; stop at the first candidate inside budget
    cands = ((1e-13, 33), (1e-13, 34), (1e-13, 36), (1e-6, 48),
             (2e-7, 48), (3e-6, 48), (1e-5, 48))
    for rtol, max_terms in cands:
        if cheapest is not None:
            break
        try:
            m = _build_model_once(F, R, rtol, max_terms=max_terms)
        except Exception:
            continue
        _attach_gains(m)
        # Scaled P/Q must be strictly positive-signed, finite, and well
        # inside fp32 range everywhere on the clamp range (the fp32 sim
        # below is the precision arbiter; this gate only guards range).
        sfine = np.linspace(-1, 1, 2000001)
        Qv = _eval_factors_scaled(m["fQ"], m["gQ"], sfine)
        Pv = _eval_factors_scaled(m["fP"], m["gP"], sfine)
        ok = (np.isfinite(Qv).all() and np.isfinite(Pv).all()
              and np.abs(Qv).min() > 1e-25 and np.abs(Qv).max() < 1e25
              and np.abs(Pv).max() < 1e25)
        if not ok:
            continue
        m["recip_mode"] = "actseed" if Qv.min() > 1e-25 else "native"
        e1 = np.abs(_eval_f32_device(m, dense).astype(np.float64) - Fd).max()
        e2 = np.abs(_eval_f32_device(m, xn).astype(np.float64) - Fn).max()
        # extra validation around the most Q-suppressed point
        s0 = sfine[int(np.argmin(Qv))]
        xloc = np.clip(s0 * R + np.linspace(-2e-4, 2e-4, 20001), -R, R)
        e3 = np.abs(_eval_f32_device(m, xloc).astype(np.float64)
                    - F(xloc)).max()
        err = max(e1, e2, e3)
        nops = sum(2 if f[0] == "q" else 1 for f in m["fP"] + m["fQ"])
        if best is None or err < best[0]:
            best = (err, nops, m)
        if err <= ACCEPT and (cheapest is None or nops < cheapest[1]):
            cheapest = (err, nops, m)
    pick = cheapest or best
    assert pick is not None, "rational model construction failed"
    pick[2]["fit_err"] = pick[0]
    return pick[2]


# ------------------------------------------------------------- bass kernel

_COMPILED = {}


def _model_key(model):
    return (model["R"], model["C"], model.get("C_dev"),
            model.get("recip_mode", "native"),
            tuple(np.round(model.get("gP", []), 12)),
            tuple(np.round(model.get("gQ", []), 12)),
            tuple((f[0], round(f[1], 14), round(f[2], 14)) if f[0] == "q"
                  else (f[0], round(f[1], 14)) for f in model["fP"]),
            tuple((f[0], round(f[1], 14), round(f[2], 14)) if f[0] == "q"
                  else (f[0], round(f[1], 14)) for f in model["fQ"]))


def _build_bass(model, rep=1, use_act=True, no_recip=False):
    """Raw-bass kernel: single [128, 977] fp32 tile per core.

    Engine plan (manual semaphores; every instruction needs at most one
    wait, which the single-wait ISA slots require):

      SP  : DMA cb in, DMA x in -> dma_sem(+16 each);
            wait dve_sem>=3+2*rep; DMA y out
      DVE : wait dma_sem>=32; cbv copy + clamp x2 (dve_sem -> 3); then per
            rep: Q-factor chain (waiting act_sem per square), +1 dve_sem
            (lets ACT start Ln), P-factor chain, Newton step from ACT's
            exp(-ln Q) seed, final stt, +1 dve_sem
      ACT : per rep: wait dve_sem (prev rep consumed), n_q Squares;
            wait dve_sem (Q done), Ln(Q), Exp(-lnQ); act_sem +1 each

    The reciprocal seed r0 = exp(-ln Q) comes from the otherwise-idle
    ScalarE (the native DVE reciprocal instruction costs ~14us/pass);
    one DVE Newton step makes it exact to fp32. The Newton form
    (Q*r0 - 2)*r0 yields -1/Q, so the final multiply uses -C.
    Requires Q > 0 on the clamp range - validated at model build
    (model["recip_mode"] == "actseed"), else native reciprocal.
    """
    from contextlib import ExitStack

    import concourse.bass as bass
    import concourse.mybir as mybir

    Alu = mybir.AluOpType
    Act = mybir.ActivationFunctionType
    f32 = mybir.dt.float32

    R = model["R"]
    C = model.get("C_dev", model["C"])
    fP, fQ = model["fP"], model["fQ"]
    gP = model.get("gP", [1.0] * len(fP))
    gQ = model.get("gQ", [1.0] * len(fQ))
    actseed = (model.get("recip_mode", "native") == "actseed") and not no_recip

    # Q chain first so ACT can compute ln/exp of Q while DVE runs P's chain.
    # Each entry: (factor, chain-index, is-first, gain)
    order = ([(f, 1, i == 0, gQ[i]) for i, f in enumerate(fQ)]
             + [(f, 0, i == 0, gP[i]) for i, f in enumerate(fP)])
    # ACT Square computes (sqrt(g)*s + sqrt(g)*h)^2 = g*(s+h)^2 via its
    # free scale/bias; the per-factor bias sqrt(g)*h is streamed in via cb.
    qparams = [(float(np.sqrt(g) / R), float(np.sqrt(g) * f[1]),
                float(g * f[2]))
               for f, _, _, g in order if f[0] == "q"]
    has_linear = any(f[0] == "l" for f, _, _, _ in order)
    hs = [hb for _, hb, _ in qparams]
    n_q = len(hs)
    acts_per_rep = n_q + (2 if actseed else 0)
    cb_host = np.tile(np.asarray(hs, np.float32), (PART, 1))

    nc = bass.Bass("TRN2", target_bir_lowering=False, debug=False,
                   num_devices=N_CORES)
    x_d = nc.dram_tensor("x", [PART, FREE], f32, kind="ExternalInput").ap()
    cb_d = nc.dram_tensor("cb", [PART, max(n_q, 1)], f32,
                          kind="ExternalInput").ap()
    y_d = nc.dram_tensor("y", [PART, FREE], f32, kind="ExternalOutput").ap()

    with ExitStack() as ctx:
        def sb(name, shape):
            return ctx.enter_context(nc.sbuf_tensor(name, shape, f32)).ap()

        xt = sb("xt", [PART, FREE])
        st = sb("st", [PART, FREE])
        sl = sb("sl", [PART, FREE])
        cb = sb("cb_s", [PART, max(n_q, 1)])
        accP = sb("accP", [PART, FREE])
        accQ = sb("accQ", [PART, FREE])
        rq = sb("rq", [PART, FREE])
        lnq = sb("lnq", [PART, FREE])
        scr = sb("scr", [PART, FREE])
        yt = sb("yt", [PART, FREE])
        sqs = [sb(f"sq{k}", [PART, FREE]) for k in range(max(n_q, 1))]
        accs = [accP, accQ]

        dma_sem = ctx.enter_context(nc.semaphore(name="dma_sem"))
        act_sem = ctx.enter_context(nc.semaphore(name="act_sem"))
        dve_sem = ctx.enter_context(nc.semaphore(name="dve_sem"))

        block = ctx.enter_context(nc.Block())

        @block.sync
        def _(sync):
            sync.dma_start(out=cb, in_=cb_d).then_inc(dma_sem, 16)
            sync.dma_start(out=xt, in_=x_d).then_inc(dma_sem, 16)
            sync.wait_ge(dve_sem, 2 + 2 * rep)
            sync.dma_start(out=y_d, in_=yt).then_inc(dma_sem, 16)

        @block.scalar
        def _(scalar):
            if not use_act:
                return
            for r in range(rep):
                # dve_sem >= 2 implies the cb DMA completed transitively
                # (DVE's first op waited on dma_sem >= 32).
                scalar.wait_ge(dve_sem, 2 + 2 * r)
                for k in range(n_q):
                    nc.scalar.activation(out=sqs[k], in_=st,
                                         func=Act.Square,
                                         bias=cb[:, k:k + 1],
                                         scale=qparams[k][0]
                                         ).then_inc(act_sem, 1)
                if actseed:
                    scalar.wait_ge(dve_sem, 3 + 2 * r)  # Q chain complete
                    nc.scalar.activation(out=lnq, in_=accQ,
                                         func=Act.Ln).then_inc(act_sem, 1)
                    nc.scalar.activation(out=rq, in_=lnq, func=Act.Exp,
                                         scale=-1.0).then_inc(act_sem, 1)

        @block.vector
        def _(vector):
            vector.wait_ge(dma_sem, 32)
            nc.vector.tensor_scalar(out=st, in0=xt, scalar1=float(R),
                                    scalar2=float(-R), op0=Alu.min,
                                    op1=Alu.max).then_inc(dve_sem, 1)
            if has_linear or not use_act:
                nc.vector.tensor_scalar(out=sl, in0=st, scalar1=1.0 / R,
                                        scalar2=None,
                                        op0=Alu.mult).then_inc(dve_sem, 1)
            else:
                nc.vector.tensor_copy(out=sl[:, :1],
                                      in_=st[:, :1]).then_inc(dve_sem, 1)
            qi = 0
            for r in range(rep):
                for oi, (f, ci, first, g) in enumerate(order):
                    acc = accs[ci]
                    is_last_q = (oi == len(fQ) - 1)
                    ins = None
                    if f[0] == "q" and use_act:
                        qi += 1
                        e = qparams[(qi - 1) % n_q][2]
                        vector.wait_ge(act_sem,
                                       qi + (2 * r if actseed else 0))
                        sq = sqs[(qi - 1) % n_q]
                        if first:
                            ins = nc.vector.tensor_scalar(
                                out=acc, in0=sq, scalar1=e, scalar2=None,
                                op0=Alu.add)
                        else:
                            ins = nc.vector.scalar_tensor_tensor(
                                out=acc, in0=sq, scalar=e, in1=acc,
                                op0=Alu.add, op1=Alu.mult)
                    elif f[0] == "q":
                        h, e = float(f[1]), float(f[2])
                        nc.vector.scalar_tensor_tensor(
                            out=scr, in0=sl, scalar=h, op0=Alu.add,
                            in1=sl, op1=Alu.mult)
                        if first:
                            ins = nc.vector.tensor_scalar(
                                out=acc, in0=scr, scalar1=e, scalar2=None,
                                op0=Alu.add)
                        else:
                            ins = nc.vector.scalar_tensor_tensor(
                                out=acc, in0=scr, scalar=e, in1=acc,
                                op0=Alu.add, op1=Alu.mult)
                    else:
                        h = float(f[1])
                        if first:
                            ins = nc.vector.tensor_scalar(
                                out=acc, in0=sl, scalar1=h, scalar2=None,
                                op0=Alu.add)
                        else:
                            ins = nc.vector.scalar_tensor_tensor(
                                out=acc, in0=sl, scalar=h, in1=acc,
                                op0=Alu.add, op1=Alu.mult)
                    if is_last_q:
                        ins.then_inc(dve_sem, 1)  # unblock ACT's Ln

                if no_recip:
                    nc.vector.tensor_scalar(
                        out=yt, in0=accP, scalar1=C, scalar2=None,
                        op0=Alu.mult).then_inc(dve_sem, 1)
                elif actseed:
                    vector.wait_ge(act_sem, (r + 1) * acts_per_rep)
                    nc.vector.tensor_tensor(scr, accQ, rq, Alu.mult)
                    nc.vector.scalar_tensor_tensor(
                        out=scr, in0=scr, scalar=2.0, op0=Alu.subtract,
                        in1=rq, op1=Alu.mult)      # scr = -1/Q
                    nc.vector.scalar_tensor_tensor(
                        out=yt, in0=accP, scalar=-C, in1=scr, op0=Alu.mult,
                        op1=Alu.mult).then_inc(dve_sem, 1)
                else:
                    nc.vector.reciprocal(out=rq, in_=accQ)
                    nc.vector.scalar_tensor_tensor(
                        out=yt, in0=accP, scalar=C, in1=rq, op0=Alu.mult,
                        op1=Alu.mult).then_inc(dve_sem, 1)

    return nc, cb_host


def _core_starts():
    starts = [c * PER_CORE for c in range(N_CORES - 1)]
    starts.append(N_TOTAL - PER_CORE)  # last core overlaps; same values
    return starts


def kernel(**inputs) -> np.ndarray:
    from concourse.bass_utils import run_bass_kernel_spmd

    x = np.ascontiguousarray(np.asarray(inputs["x_t"], np.float32))
    assert x.shape == (N_TOTAL, 1), x.shape
    weights = {k: np.asarray(v, np.float64) for k, v in inputs.items()
               if k != "x_t"}

    model = build_model(weights)
    key = _model_key(model)
    if key not in _COMPILED:
        _COMPILED.clear()
        _COMPILED[key] = _build_bass(model)
    nc, cb_host = _COMPILED[key]

    xf = x.reshape(-1)
    starts = _core_starts()
    in_maps = [{"x": xf[s:s + PER_CORE].reshape(PART, FREE).copy(),
                "cb": cb_host}
               for s in starts]
    res = run_bass_kernel_spmd(nc, in_maps, core_ids=list(range(N_CORES)))
    out = np.empty(N_TOTAL, np.float32)
    for s, r in zip(starts, res.results):
        out[s:s + PER_CORE] = np.asarray(r["y"], np.float32).reshape(-1)
    return out


if __name__ == "__main__":
    rng = np.random.default_rng(0)
    fake = {
        "x_t": rng.normal(size=(N_TOTAL, 1)).astype(np.float32),
        "W1": (rng.normal(size=(_H, 1)) * 0.1).astype(np.float32),
        "b1": (rng.normal(size=(_H,)) * 0.1).astype(np.float32),
        "Wih": (rng.normal(size=(_L, 4 * _H, _H)) * 0.1).astype(np.float32),
        "Whh": (rng.normal(size=(_L, 4 * _H, _H)) * 0.1).astype(np.float32),
        "b_ih": (rng.normal(size=(_L, 4 * _H)) * 0.1).astype(np.float32),
        "b_hh": (rng.normal(size=(_L, 4 * _H)) * 0.1).astype(np.float32),
        "g_x": (1 + rng.normal(size=(_L, 4 * _H)) * 0.1).astype(np.float32),
        "be_x": (rng.normal(size=(_L, 4 * _H)) * 0.1).astype(np.float32),
        "g_h": (1 + rng.normal(size=(_L, 4 * _H)) * 0.1).astype(np.float32),
        "be_h": (rng.normal(size=(_L, 4 * _H)) * 0.1).astype(np.float32),
        "g_c": (1 + rng.normal(size=(_L, _H)) * 0.1).astype(np.float32),
        "be_c": (rng.normal(size=(_L, _H)) * 0.1).astype(np.float32),
        "Wo": (rng.normal(size=(1, _H)) * 0.1).astype(np.float32),
        "bo": (rng.normal(size=(1,)) * 0.1).astype(np.float32),
    }
    out = kernel(**fake)
    exp = _ref_np(**{k: np.asarray(v, np.float64) for k, v in fake.items()})
    err = np.abs(out - exp).max()
    print("self-test max abs err:", err)
